# revision 1
# baseline (speedup 1.0000x reference)
"""Trainium2 Bass kernel for nn_MessagePassingGNN (B=8192 graphs, N=9 nodes,
16 edges + 9 self-loops per graph, 4 message-passing steps + GRU, decoder).

Strategy:
  - Data-parallel over batch: each of 8 cores gets 1024 graphs.
  - Within a core, graphs are packed into blocks of 14 (126 nodes, 350 edges)
    plus one tail block of 2 graphs, processed as 37 block-pairs. Gather
    (x[dst], x[src]) and scatter (mean-aggregation) are matmuls against
    host-precomputed one-hot incidence matrices, so the whole step pipeline
    lives on the TensorEngine.
  - No transposes anywhere: W1 is applied FIRST in node space (P = X @ W1,
    with x kept transposed [feat, nodes] so x itself is the stationary
    operand), and the gather then accumulates P halves in edge space. W3 is
    emitted in row form (stationary = m2^T slices) so the scatter gets its
    edge-major operand for free.
  - All matmuls bf16 (fp32 PSUM accumulation). Sigmoid is rewritten via tanh
    (z = 0.5*(1+tanh(g/2))) so ScalarE needs a single LUT table set.
  - deg-normalization is folded into the scatter one-hot; msg_b3 is folded
    into the GRU input bias (host-side), so no partition-broadcast is needed.
  - Encoder/decoder are per-pair (no serial phases); WAVE=6 pairs are emitted
    phase-interleaved so each engine's in-order stream has independent work
    to fill cross-engine dependency stalls; PSUM is rotated as 4x 1-bank +
    2x 2-bank pool slots.
"""

import numpy as np

try:
    import concourse.bass as bass  # noqa: F401
except Exception:  # pragma: no cover
    import sys

    sys.path.insert(0, "/opt/trn_rl_repo")

import ml_dtypes
import concourse.bass as bass
import concourse.bacc as bacc
import concourse.mybir as mybir
from concourse.bass import MemorySpace
from concourse.bass_utils import run_bass_kernel_spmd
from concourse.tile import TileContext

BF16 = mybir.dt.bfloat16
F32 = mybir.dt.float32
NPBF16 = ml_dtypes.bfloat16
AF = mybir.ActivationFunctionType
ALU = mybir.AluOpType

N, F_IN, H, MH, STEPS = 9, 15, 128, 256, 4
E_PER = 16
EPG = E_PER + N  # 25 edges per graph incl self-loops
NCORES = 8
GPB = 14  # graphs per full block
NN = GPB * N  # 126 nodes per full block
NE = GPB * EPG  # 350 edges per full block

# bias-pack column map
COL_ENC = 0
COL_B1 = lambda s, c: 1 + 2 * s + c
COL_B2 = lambda s, c: 9 + 2 * s + c
COL_BRZ = lambda s, g: 17 + 2 * s + g  # 0.5*(bi'+bh)[g*128:+128]
COL_BHN = lambda s: 25 + s  # bh[256:384]
COL_BIN = lambda s: 29 + s  # bi'[256:384]
COL_DB1 = lambda c: 33 + c
COL_DB2 = lambda c: 35 + c
COL_DB3 = 37
NBIAS = 38


def _derive(bg):
    nblk = bg // GPB
    tailg = bg - nblk * GPB
    totblk = nblk + (1 if tailg else 0)
    nnode = bg * N
    return dict(bg=bg, nblk=nblk, tailg=tailg, totblk=totblk, nnode=nnode)


CFG_FULL = _derive(1024)

_NC_CACHE = {}


WAVE = 6       # pairs interleaved per wave
SD_BUFS = 14
XP_BUFS = 38
ACT_BUFS = 6
PB_BUFS = 4
PB2_BUFS = 2


def build_nc(cfg, repeat=1):
    key = (cfg["bg"], repeat, WAVE, SD_BUFS, XP_BUFS, ACT_BUFS, PB_BUFS, PB2_BUFS)
    if key in _NC_CACHE:
        return _NC_CACHE[key]
    nblk, tailg, totblk, nnode = (
        cfg["nblk"],
        cfg["tailg"],
        cfg["totblk"],
        cfg["nnode"],
    )
    tnn, tne = tailg * N, tailg * EPG

    nc = bacc.Bacc("TRN2", target_bir_lowering=False, debug=False, num_devices=NCORES)

    obsT_d = nc.dram_tensor("obsT", [F_IN, nnode], BF16, kind="ExternalInput")
    sdt_d = nc.dram_tensor("sdt", [totblk, NN, 2 * NE], BF16, kind="ExternalInput")
    dwt_d = nc.dram_tensor("dwt", [totblk, 3, 128, NN], BF16, kind="ExternalInput")
    encw_d = nc.dram_tensor("encw", [F_IN, H], BF16, kind="ExternalInput")
    w1_d = nc.dram_tensor("w1", [STEPS, 2 * H, MH], BF16, kind="ExternalInput")
    w2_d = nc.dram_tensor("w2", [STEPS, MH, MH], BF16, kind="ExternalInput")
    w3_d = nc.dram_tensor("w3", [STEPS, MH, H], BF16, kind="ExternalInput")
    wi_d = nc.dram_tensor("wi", [STEPS, H, 3 * H], BF16, kind="ExternalInput")
    wh_d = nc.dram_tensor("wh", [STEPS, H, 3 * H], BF16, kind="ExternalInput")
    dw1_d = nc.dram_tensor("dw1", [H, MH], BF16, kind="ExternalInput")
    dw2_d = nc.dram_tensor("dw2", [MH, MH], BF16, kind="ExternalInput")
    dw3_d = nc.dram_tensor("dw3", [MH, 1], BF16, kind="ExternalInput")
    bias_d = nc.dram_tensor("biases", [128, NBIAS], F32, kind="ExternalInput")
    out_d = nc.dram_tensor("out", [1, nnode], F32, kind="ExternalOutput")

    NN2 = 2 * NN

    with TileContext(nc) as tc:
        with (
            tc.tile_pool(name="const", bufs=1) as constp,
            tc.tile_pool(name="sd", bufs=SD_BUFS) as sdp,
            tc.tile_pool(name="dw", bufs=SD_BUFS) as dwp,
            tc.tile_pool(name="xp", bufs=XP_BUFS) as xpp,
            tc.tile_pool(name="eact", bufs=ACT_BUFS) as eactp,
            tc.tile_pool(name="gact", bufs=ACT_BUFS) as gactp,
            tc.tile_pool(name="pb", bufs=PB_BUFS, space=MemorySpace.PSUM) as ppb,
            tc.tile_pool(name="pb2", bufs=PB2_BUFS, space=MemorySpace.PSUM) as ppb2,
        ):
            obs_t = constp.tile([F_IN, nnode], BF16, tag="obs")
            nc.sync.dma_start(obs_t[:], obsT_d[:])
            encw_t = constp.tile([F_IN, H], BF16, tag="encw")
            nc.sync.dma_start(encw_t[:], encw_d[:])
            w1_t = constp.tile([128, STEPS, 2, MH], BF16, tag="w1")
            nc.sync.dma_start(
                w1_t[:], w1_d.rearrange("s (kc p) m -> p s kc m", p=128)
            )
            w2_t = constp.tile([128, STEPS, 2, MH], BF16, tag="w2")
            nc.sync.dma_start(
                w2_t[:], w2_d.rearrange("s (kc p) m -> p s kc m", p=128)
            )
            w3_t = constp.tile([128, STEPS, 2, H], BF16, tag="w3")
            nc.sync.dma_start(
                w3_t[:], w3_d.rearrange("s (kc p) m -> p s kc m", p=128)
            )
            wi_t = constp.tile([128, STEPS, 3 * H], BF16, tag="wi")
            nc.sync.dma_start(wi_t[:], wi_d.rearrange("s p m -> p s m"))
            wh_t = constp.tile([128, STEPS, 3 * H], BF16, tag="wh")
            nc.sync.dma_start(wh_t[:], wh_d.rearrange("s p m -> p s m"))
            dw1_t = constp.tile([128, MH], BF16, tag="dw1")
            nc.sync.dma_start(dw1_t[:], dw1_d[:])
            dw2_t = constp.tile([128, 2, MH], BF16, tag="dw2")
            nc.sync.dma_start(dw2_t[:], dw2_d.rearrange("(kc p) m -> p kc m", p=128))
            dw3_t = constp.tile([128, 2, 1], BF16, tag="dw3")
            nc.sync.dma_start(dw3_t[:], dw3_d.rearrange("(kc p) m -> p kc m", p=128))
            bias_t = constp.tile([128, NBIAS], F32, tag="bias")
            nc.sync.dma_start(bias_t[:], bias_d[:])

            def bcol(c):
                return bias_t[:, c : c + 1]

            tot = cfg["totblk"]
            pairs = [tuple(range(kk, min(kk + 2, tot))) for kk in range(0, tot, 2)]

            def geom(k):
                full = k < nblk
                nn = NN if full else tnn
                ne = NE if full else tne
                ecs = [(0, 128), (128, 128), (256, 94)] if full else [(0, tne)]
                return nn, ne, ecs

            class Ctx:
                pass

            def ph_load(cx):
                cx.sds, cx.dws = [], []
                for bi, k in enumerate(cx.pr):
                    nn, ne, ecs = cx.geos[bi]
                    sd = sdp.tile([NN, 2 * NE], BF16, tag="sd", name="sd")
                    if k < nblk:
                        nc.sync.dma_start(sd[:, :], sdt_d[k])
                    else:
                        nc.sync.dma_start(sd[:nn, :ne], sdt_d[k, :nn, :ne])
                        nc.sync.dma_start(
                            sd[:nn, NE : NE + ne], sdt_d[k, :nn, NE : NE + ne]
                        )
                    dwti = dwp.tile([128, 3, NN], BF16, tag="dw", name="dw")
                    nch = len(ecs)
                    nc.sync.dma_start(
                        dwti[:, :nch, :nn],
                        dwt_d[k, :nch, :, :nn].rearrange("c p f -> p c f"),
                    )
                    cx.sds.append(sd)
                    cx.dws.append(dwti)

            def ph_enc(cx):
                penc = ppb.tile([128, 512], F32, tag="pb", name="penc")
                nc.tensor.matmul(
                    penc[:, : cx.npair], encw_t[:, :], obs_t[:, cx.pcols],
                    start=True, stop=True,
                )
                cx.xcur = xpp.tile([128, NN2], BF16, tag="xp", name="x0")
                nc.scalar.activation(
                    cx.xcur[:, : cx.npair], penc[:, : cx.npair], AF.Tanh,
                    bias=bcol(COL_ENC),
                )

            def ph_P(cx, s):
                # P = x @ W1-halves in node space (row layout), then to SBUF
                cx.psb = eactp.tile(
                    [128, 2, 2, 2, 128], BF16, tag="psb", name="psb"
                )  # [part, blk, h, mc, feat]
                for bi in range(len(cx.pr)):
                    nn = cx.geos[bi][0]
                    c0 = NN * bi
                    pq = ppb.tile([128, 512], F32, tag="pb", name="pq")
                    for h in range(2):
                        for mc in range(2):
                            o = 256 * h + 128 * mc
                            nc.tensor.matmul(
                                pq[:nn, o : o + 128],
                                cx.xcur[:, c0 : c0 + nn],
                                w1_t[:, s, h, mc * 128 : mc * 128 + 128],
                                start=True, stop=True,
                            )
                    nc.vector.tensor_copy(cx.psb[:126, bi, :, :, :], pq[:126, :])

            def ph_m1(cx, s):
                cx.m1sb = eactp.tile(
                    [128, 2, 2, NE], BF16, tag="m1", name="m1sb"
                )  # [part, mc, blk, edge]
                for mc in range(2):
                    pm = ppb2.tile([128, 1024], F32, tag="pb2", name="pm")
                    for bi in range(len(cx.pr)):
                        nn, ne, _ = cx.geos[bi]
                        o = 512 * bi
                        nc.tensor.matmul(
                            pm[:, o : o + ne],
                            cx.psb[:nn, bi, 0, mc, :],
                            cx.sds[bi][:nn, :ne],
                            start=True, stop=False,
                        )
                        nc.tensor.matmul(
                            pm[:, o : o + ne],
                            cx.psb[:nn, bi, 1, mc, :],
                            cx.sds[bi][:nn, NE : NE + ne],
                            start=False, stop=True,
                        )
                    if cx.uni:
                        ne = cx.geos[0][1]
                        nc.scalar.activation(
                            cx.m1sb[:, mc, :, :ne],
                            pm.rearrange("p (b f) -> p b f", b=2)[:, :, :ne],
                            AF.Tanh, bias=bcol(COL_B1(s, mc)),
                        )
                    else:
                        for bi in range(len(cx.pr)):
                            ne = cx.geos[bi][1]
                            nc.scalar.activation(
                                cx.m1sb[:, mc, bi, :ne],
                                pm[:, 512 * bi : 512 * bi + ne],
                                AF.Tanh, bias=bcol(COL_B1(s, mc)),
                            )

            def ph_m2(cx, s):
                cx.m2sb = eactp.tile([128, 2, 2, NE], BF16, tag="m2", name="m2sb")
                for mc in range(2):
                    pm = ppb2.tile([128, 1024], F32, tag="pb2", name="pm2")
                    for bi in range(len(cx.pr)):
                        nn, ne, _ = cx.geos[bi]
                        o = 512 * bi
                        for kc in range(2):
                            nc.tensor.matmul(
                                pm[:, o : o + ne],
                                w2_t[:, s, kc, mc * 128 : mc * 128 + 128],
                                cx.m1sb[:, kc, bi, :ne],
                                start=(kc == 0), stop=(kc == 1),
                            )
                    if cx.uni:
                        ne = cx.geos[0][1]
                        nc.scalar.activation(
                            cx.m2sb[:, mc, :, :ne],
                            pm.rearrange("p (b f) -> p b f", b=2)[:, :, :ne],
                            AF.Tanh, bias=bcol(COL_B2(s, mc)),
                        )
                    else:
                        for bi in range(len(cx.pr)):
                            ne = cx.geos[bi][1]
                            nc.scalar.activation(
                                cx.m2sb[:, mc, bi, :ne],
                                pm[:, 512 * bi : 512 * bi + ne],
                                AF.Tanh, bias=bcol(COL_B2(s, mc)),
                            )

            def ph_w3(cx, s):
                # W3 row-form + scatter into one pair tile:
                # per block bi at 512*bi: m3 chunks [0:384], aggr [384:384+nn]
                cx.m3sb = eactp.tile([128, 2, 3, 128], BF16, tag="m3r", name="m3sb")
                cx.aggp = gactp.tile([128, NN2], BF16, tag="aggr", name="aggp")
                aoff = 0
                for bi in range(len(cx.pr)):
                    nn, _, ecs = cx.geos[bi]
                    pg3 = ppb.tile([128, 512], F32, tag="pb", name="pg3")
                    for ci, (e0, el) in enumerate(ecs):
                        for kc in range(2):
                            nc.tensor.matmul(
                                pg3[:el, 128 * ci : 128 * ci + 128],
                                cx.m2sb[:, kc, bi, e0 : e0 + el],
                                w3_t[:, s, kc, :],
                                start=(kc == 0), stop=(kc == 1),
                            )
                    nch = len(ecs)
                    nc.vector.tensor_copy(
                        cx.m3sb[:, bi, :nch, :], pg3[:, : 128 * nch]
                    )
                    for ci, (e0, el) in enumerate(ecs):
                        nc.tensor.matmul(
                            pg3[:, 384 : 384 + nn],
                            cx.m3sb[:el, bi, ci, :],
                            cx.dws[bi][:el, ci, :nn],
                            start=(ci == 0), stop=(ci == len(ecs) - 1),
                        )
                    nc.vector.tensor_copy(
                        cx.aggp[:, aoff : aoff + nn], pg3[:, 384 : 384 + nn]
                    )
                    aoff += nn

            def ph_gru(cx, s):
                npair = cx.npair
                pgr = ppb.tile([128, 512], F32, tag="pb", name="pgr")
                pgn = ppb.tile([128, 512], F32, tag="pb", name="pgn")
                # pgr: rz0@0, rz1@npair; pgn: gin@0, ghn@npair
                for g, off in ((0, 0), (1, npair)):
                    nc.tensor.matmul(
                        pgr[:, off : off + npair],
                        wi_t[:, s, g * 128 : g * 128 + 128],
                        cx.aggp[:, :npair],
                        start=True, stop=False,
                    )
                    nc.tensor.matmul(
                        pgr[:, off : off + npair],
                        wh_t[:, s, g * 128 : g * 128 + 128],
                        cx.xcur[:, :npair],
                        start=False, stop=True,
                    )
                nc.tensor.matmul(
                    pgn[:, :npair],
                    wi_t[:, s, 256:384],
                    cx.aggp[:, :npair],
                    start=True, stop=True,
                )
                nc.tensor.matmul(
                    pgn[:, npair : 2 * npair],
                    wh_t[:, s, 256:384],
                    cx.xcur[:, :npair],
                    start=True, stop=True,
                )
                thr = gactp.tile([128, NN2], BF16, tag="thr", name="thr")
                cx.thz = gactp.tile([128, NN2], BF16, tag="thz", name="thz")
                nc.scalar.activation(
                    thr[:, :npair], pgr[:, :npair], AF.Tanh,
                    bias=bcol(COL_BRZ(s, 0)), scale=0.5,
                )
                nc.scalar.activation(
                    cx.thz[:, :npair], pgr[:, npair : 2 * npair], AF.Tanh,
                    bias=bcol(COL_BRZ(s, 1)), scale=0.5,
                )
                hnp = gactp.tile([128, NN2], BF16, tag="hnp", name="hnp")
                nc.vector.tensor_scalar(
                    hnp[:, :npair],
                    pgn[:, npair : 2 * npair],
                    bcol(COL_BHN(s)), 0.5,
                    op0=ALU.add, op1=ALU.mult,
                )
                rhn = gactp.tile([128, NN2], BF16, tag="rhn", name="rhn")
                nc.vector.scalar_tensor_tensor(
                    rhn[:, :npair], thr[:, :npair], 1.0, hnp[:, :npair],
                    op0=ALU.add, op1=ALU.mult,
                )
                cx.tn = gactp.tile([128, NN2], BF16, tag="tn", name="tn")
                nc.vector.scalar_tensor_tensor(
                    cx.tn[:, :npair], pgn[:, :npair],
                    bcol(COL_BIN(s)), rhn[:, :npair],
                    op0=ALU.add, op1=ALU.add,
                )

            def ph_xupd(cx, s):
                npair = cx.npair
                ngate = gactp.tile([128, NN2], BF16, tag="ng", name="ng")
                nc.scalar.activation(ngate[:, :npair], cx.tn[:, :npair], AF.Tanh)
                # x' = n + z*(x-n), z = 0.5*(1+th_z)
                d_ = gactp.tile([128, NN2], BF16, tag="d", name="d_")
                nc.gpsimd.tensor_sub(
                    d_[:, :npair], cx.xcur[:, :npair], ngate[:, :npair]
                )
                w_ = gactp.tile([128, NN2], BF16, tag="w", name="w_")
                nc.vector.scalar_tensor_tensor(
                    w_[:, :npair], cx.thz[:, :npair], 1.0, d_[:, :npair],
                    op0=ALU.add, op1=ALU.mult,
                )
                xnxt = xpp.tile([128, NN2], BF16, tag="xp", name="xn")
                nc.vector.scalar_tensor_tensor(
                    xnxt[:, :npair], w_[:, :npair], 0.5, ngate[:, :npair],
                    op0=ALU.mult, op1=ALU.add,
                )
                cx.xcur = xnxt

            def ph_dec1(cx):
                npair = cx.npair
                pd1 = ppb.tile([128, 512], F32, tag="pb", name="pd1")
                cx.d1sb = gactp.tile([128, 2, NN2], BF16, tag="d1", name="d1sb")
                for mc in range(2):
                    nc.tensor.matmul(
                        pd1[:, 252 * mc : 252 * mc + npair],
                        dw1_t[:, mc * 128 : mc * 128 + 128],
                        cx.xcur[:, :npair],
                        start=True, stop=True,
                    )
                    nc.scalar.activation(
                        cx.d1sb[:, mc, :npair],
                        pd1[:, 252 * mc : 252 * mc + npair],
                        AF.Tanh, bias=bcol(COL_DB1(mc)),
                    )

            def ph_dec2(cx):
                npair = cx.npair
                pd2 = ppb.tile([128, 512], F32, tag="pb", name="pd2")
                d2sb = gactp.tile([128, 2, NN2], BF16, tag="d2", name="d2sb")
                for mc in range(2):
                    for kc in range(2):
                        nc.tensor.matmul(
                            pd2[:, 252 * mc : 252 * mc + npair],
                            dw2_t[:, kc, mc * 128 : mc * 128 + 128],
                            cx.d1sb[:, kc, :npair],
                            start=(kc == 0), stop=(kc == 1),
                        )
                    nc.scalar.activation(
                        d2sb[:, mc, :npair],
                        pd2[:, 252 * mc : 252 * mc + npair],
                        AF.Tanh, bias=bcol(COL_DB2(mc)),
                    )
                pd3 = ppb.tile([128, 512], F32, tag="pb", name="pd3")
                for kc in range(2):
                    nc.tensor.matmul(
                        pd3[:1, :npair], dw3_t[:, kc, :], d2sb[:, kc, :npair],
                        start=(kc == 0), stop=(kc == 1),
                    )
                outp = gactp.tile([1, NN2], F32, tag="outp", name="outp")
                nc.scalar.activation(
                    outp[:, :npair], pd3[:1, :npair], AF.Identity,
                    bias=bias_t[0:1, COL_DB3 : COL_DB3 + 1],
                )
                nc.sync.dma_start(out_d[:, cx.pcols], outp[:1, :npair])

            for _rep in range(repeat):
                allpairs = list(pairs)
                waves = [
                    allpairs[i : i + WAVE] for i in range(0, len(allpairs), WAVE)
                ]
                for wv in waves:
                    cxs = []
                    for pr in wv:
                        cx = Ctx()
                        cx.pr = pr
                        cx.geos = [geom(k) for k in pr]
                        cx.uni = len(pr) == 2 and cx.geos[0] == cx.geos[1]
                        cx.col0 = NN * pr[0]
                        cx.npair = sum(g[0] for g in cx.geos)
                        cx.pcols = slice(cx.col0, cx.col0 + cx.npair)
                        cxs.append(cx)
                    for cx in cxs:
                        ph_load(cx)
                    for cx in cxs:
                        ph_enc(cx)
                    for s in range(STEPS):
                        for ph in (ph_P, ph_m1, ph_m2, ph_w3, ph_gru, ph_xupd):
                            for cx in cxs:
                                ph(cx, s)
                    for cx in cxs:
                        ph_dec1(cx)
                    for cx in cxs:
                        ph_dec2(cx)

    nc.compile()
    _NC_CACHE[key] = nc
    return nc


def bench_hw(inputs, repeats=(1, 3), n_iter=14):
    """Differential HW timing: wall-clock difference between NEFFs that run
    the message-passing phase R times (dispatch overhead cancels)."""
    from test import bench  # local harness helper

    in_maps = preprocess(inputs, CFG_FULL)
    res = {}
    for r in repeats:
        nc = build_nc(CFG_FULL, repeat=r)
        times, _ = bench(nc, in_maps, n_iter=n_iter)
        ts = np.sort(times)[: max(3, n_iter // 2)]
        res[r] = ts.mean()
        print(f"repeat={r}: min {times.min()*1e3:.3f} ms  "
              f"low-half-mean {ts.mean()*1e3:.3f} ms")
    rs = sorted(res)
    phase = (res[rs[-1]] - res[rs[0]]) / (rs[-1] - rs[0])
    print(f"block-phase time ≈ {phase*1e3:.3f} ms")
    return phase, res


def preprocess(inputs, cfg):
    bg, nblk, tailg, totblk, nnode = (
        cfg["bg"], cfg["nblk"], cfg["tailg"], cfg["totblk"], cfg["nnode"],
    )
    b = bg * NCORES
    obs = np.asarray(inputs["obs"], np.float32)
    edges = np.asarray(inputs["edges"], np.int64)

    # one-hot incidence per graph
    src = edges[:, 0, :]
    dst = edges[:, 1, :]
    loops = np.broadcast_to(np.arange(N, dtype=np.int64), (b, N))
    src_all = np.concatenate([src, loops], 1)  # [b, 25]
    dst_all = np.concatenate([dst, loops], 1)
    nod = np.arange(N, dtype=np.int64)
    Sg = (src_all[:, None, :] == nod[None, :, None]).astype(np.float32)  # [b,9,25]
    Dg = (dst_all[:, None, :] == nod[None, :, None]).astype(np.float32)  # [b,9,25]
    deg = Dg.sum(2)  # [b, 9] >= 1
    Dw = Dg.transpose(0, 2, 1) / deg[:, None, :]  # [b, 25, 9]

    SDt = np.zeros((NCORES, totblk, NN, 2 * NE), NPBF16)
    DWf = np.zeros((NCORES, totblk, 384, NN), np.float32)
    Sg_ = Sg.reshape(NCORES, bg, N, EPG)
    Dg_ = Dg.reshape(NCORES, bg, N, EPG)
    Dw_ = Dw.reshape(NCORES, bg, EPG, N)
    nmain = nblk * GPB
    Sm = Sg_[:, :nmain].reshape(NCORES, nblk, GPB, N, EPG)
    Dm = Dg_[:, :nmain].reshape(NCORES, nblk, GPB, N, EPG)
    Wm = Dw_[:, :nmain].reshape(NCORES, nblk, GPB, EPG, N)
    for i in range(GPB):
        r = slice(N * i, N * i + N)
        c = slice(EPG * i, EPG * i + EPG)
        SDt[:, :nblk, r, c] = Dm[:, :, i]  # dst-gather one-hot
        SDt[:, :nblk, r, NE + EPG * i : NE + EPG * i + EPG] = Sm[:, :, i]
        DWf[:, :nblk, c, r] = Wm[:, :, i]
    for i in range(tailg):
        g = nmain + i
        r = slice(N * i, N * i + N)
        c = slice(EPG * i, EPG * i + EPG)
        SDt[:, nblk, r, c] = Dg_[:, g]
        SDt[:, nblk, r, NE + EPG * i : NE + EPG * i + EPG] = Sg_[:, g]
        DWf[:, nblk, c, r] = Dw_[:, g]
    DWt = DWf.reshape(NCORES, totblk, 3, 128, NN).astype(NPBF16)

    obsT = (
        obs.reshape(b, N, F_IN)
        .reshape(NCORES, nnode, F_IN)
        .transpose(0, 2, 1)
        .astype(NPBF16)
    )  # [8, 15, nnode]

    f32 = lambda x: np.asarray(x, np.float32)
    bf = lambda x: np.ascontiguousarray(f32(x)).astype(NPBF16)

    biases = np.zeros((128, NBIAS), np.float32)
    biases[:, COL_ENC] = f32(inputs["enc_b"])
    gru_bi = f32(inputs["gru_bi"])
    gru_bh = f32(inputs["gru_bh"])
    msg_b3 = f32(inputs["msg_b3"])
    gru_Wi = f32(inputs["gru_Wi"])
    for s in range(STEPS):
        b1 = f32(inputs["msg_b1"][s])
        b2 = f32(inputs["msg_b2"][s])
        for c in range(2):
            biases[:, COL_B1(s, c)] = b1[128 * c : 128 * (c + 1)]
            biases[:, COL_B2(s, c)] = b2[128 * c : 128 * (c + 1)]
        bip = gru_bi[s] + msg_b3[s] @ gru_Wi[s]  # fold msg_b3 into GRU input bias
        for g in range(2):
            biases[:, COL_BRZ(s, g)] = 0.5 * (
                bip[128 * g : 128 * (g + 1)] + gru_bh[s][128 * g : 128 * (g + 1)]
            )
        biases[:, COL_BHN(s)] = gru_bh[s][256:384]
        biases[:, COL_BIN(s)] = bip[256:384]
    db1 = f32(inputs["dec_b1"])
    db2 = f32(inputs["dec_b2"])
    for c in range(2):
        biases[:, COL_DB1(c)] = db1[128 * c : 128 * (c + 1)]
        biases[:, COL_DB2(c)] = db2[128 * c : 128 * (c + 1)]
    biases[0, COL_DB3] = float(f32(inputs["dec_b3"])[0])

    shared = dict(
        encw=bf(inputs["enc_W"]),
        w1=bf(inputs["msg_W1"]),
        w2=bf(inputs["msg_W2"]),
        w3=bf(inputs["msg_W3"]),
        wi=bf(inputs["gru_Wi"]),
        wh=bf(inputs["gru_Wh"]),
        dw1=bf(inputs["dec_W1"]),
        dw2=bf(inputs["dec_W2"]),
        dw3=bf(inputs["dec_W3"]),
        biases=biases,
    )
    in_maps = []
    for c in range(NCORES):
        m = dict(shared)
        m["obsT"] = np.ascontiguousarray(obsT[c])
        m["sdt"] = np.ascontiguousarray(SDt[c])
        m["dwt"] = np.ascontiguousarray(DWt[c])
        in_maps.append(m)
    return in_maps


LAST_EXEC_NS = None
TRACE = False


def _run(inputs, cfg):
    global LAST_EXEC_NS
    nc = build_nc(cfg)
    in_maps = preprocess(inputs, cfg)
    res = run_bass_kernel_spmd(
        nc, in_maps, core_ids=list(range(NCORES)), trace=TRACE
    )
    LAST_EXEC_NS = res.exec_time_ns
    bg = cfg["bg"]
    outs = [np.asarray(res.results[c]["out"], np.float32).reshape(bg, N) for c in range(NCORES)]
    full = np.concatenate(outs, 0)  # [B, 9]
    return np.ascontiguousarray(full[:, :8])


def kernel(**inputs) -> np.ndarray:
    return _run(inputs, CFG_FULL)



# revision 19
# speedup vs baseline: 2.3230x; 2.3230x over previous
"""Trainium2 Bass kernel for nn_MessagePassingGNN (B=8192 graphs, N=9 nodes,
16 edges + 9 self-loops per graph, 4 message-passing steps + GRU, decoder).

Strategy:
  - Data-parallel over batch: each of 8 cores gets 1024 graphs.
  - Within a core, graphs are packed into blocks of 14 (126 nodes, 350 edges)
    plus one tail block of 2 graphs, processed as 37 block-pairs. Gather
    (x[dst], x[src]) and scatter (mean-aggregation) are matmuls against
    host-precomputed one-hot incidence matrices, so the whole step pipeline
    lives on the TensorEngine.
  - No transposes anywhere: W1 is applied FIRST in node space (P = X @ W1,
    with x kept transposed [feat, nodes] so x itself is the stationary
    operand), and the gather then accumulates P halves in edge space. W3 is
    emitted in row form (stationary = m2^T slices) so the scatter gets its
    edge-major operand for free.
  - Node-space matmuls (encoder, P, GRU, decoder, scatter) are bf16; the
    edge-space pipeline (gather, m2, m3) runs fp8-e4m3 with DoubleRow perf
    mode: one matmul contracts 2x128 rows (dst+src halves for the gather,
    both 128-chunks of the 256-wide contraction for m2/m3), halving
    TensorE cycles there. One-hot incidence entries are exact in fp8.
    Sigmoid is rewritten via tanh (z = 0.5*(1+tanh(g/2))) so ScalarE needs
    a single LUT table set.
  - deg-normalization is folded into the scatter one-hot; msg_b3 is folded
    into the GRU input bias (host-side), so no partition-broadcast is needed.
  - Encoder/decoder are per-pair (no serial phases); WAVE=6 pairs are emitted
    phase-interleaved so each engine's in-order stream has independent work
    to fill cross-engine dependency stalls; PSUM is rotated as 4x 1-bank +
    2x 2-bank pool slots.
"""

import numpy as np

try:
    import concourse.bass as bass  # noqa: F401
except Exception:  # pragma: no cover
    import sys

    sys.path.insert(0, "/opt/trn_rl_repo")

import ml_dtypes
import concourse.bass as bass
import concourse.bacc as bacc
import concourse.mybir as mybir
from concourse.bass import MemorySpace
from concourse.bass_utils import run_bass_kernel_spmd
from concourse.tile import TileContext

BF16 = mybir.dt.bfloat16
F32 = mybir.dt.float32
E4 = mybir.dt.float8e4
NPBF16 = ml_dtypes.bfloat16
NPE4 = ml_dtypes.float8_e4m3
AF = mybir.ActivationFunctionType
ALU = mybir.AluOpType
DR = mybir.MatmulPerfMode.DoubleRow

N, F_IN, H, MH, STEPS = 9, 15, 128, 256, 4
E_PER = 16
EPG = E_PER + N  # 25 edges per graph incl self-loops
NCORES = 8
GPB = 14  # graphs per full block
NN = GPB * N  # 126 nodes per full block
NE = GPB * EPG  # 350 edges per full block
NEP = 352  # NE padded so fp8 DoubleRow group strides are 16B-aligned

# bias-pack column map
COL_ENC = 0
COL_B1 = lambda s, c: 1 + 2 * s + c
COL_B2 = lambda s, c: 9 + 2 * s + c
COL_BRZ = lambda s, g: 17 + 2 * s + g  # 0.5*(bi'+bh)[g*128:+128]
COL_BHN = lambda s: 25 + s  # bh[256:384]
COL_BIN = lambda s: 29 + s  # bi'[256:384]
COL_DB1 = lambda c: 33 + c
COL_DB2 = lambda c: 35 + c
COL_DB3 = 37
NBIAS = 38


def _derive(bg):
    nblk = bg // GPB
    tailg = bg - nblk * GPB
    totblk = nblk + (1 if tailg else 0)
    nnode = bg * N
    return dict(bg=bg, nblk=nblk, tailg=tailg, totblk=totblk, nnode=nnode)


CFG_FULL = _derive(1024)

_NC_CACHE = {}


WAVE = 6       # pairs interleaved per wave
SD_BUFS = 14
XP_BUFS = 38
ACT_BUFS = 6
PB_BUFS = 4
PB2_BUFS = 2


def build_nc(cfg, repeat=1):
    key = (cfg["bg"], repeat, WAVE, SD_BUFS, XP_BUFS, ACT_BUFS, PB_BUFS, PB2_BUFS)
    if key in _NC_CACHE:
        return _NC_CACHE[key]
    nblk, tailg, totblk, nnode = (
        cfg["nblk"],
        cfg["tailg"],
        cfg["totblk"],
        cfg["nnode"],
    )
    tnn, tne = tailg * N, tailg * EPG

    nc = bacc.Bacc("TRN2", target_bir_lowering=False, debug=False, num_devices=NCORES)

    obsT_d = nc.dram_tensor("obsT", [F_IN, nnode], BF16, kind="ExternalInput")
    sdt_d = nc.dram_tensor("sdt", [totblk, NN, 2, NEP], E4, kind="ExternalInput")
    dwt_d = nc.dram_tensor("dwt", [totblk, 3, 128, NN], BF16, kind="ExternalInput")
    encw_d = nc.dram_tensor("encw", [F_IN, H], BF16, kind="ExternalInput")
    w1_d = nc.dram_tensor("w1", [STEPS, 2 * H, MH], BF16, kind="ExternalInput")
    w2_d = nc.dram_tensor("w2", [STEPS, MH, MH], E4, kind="ExternalInput")
    w3_d = nc.dram_tensor("w3", [STEPS, MH, H], E4, kind="ExternalInput")
    wi_d = nc.dram_tensor("wi", [STEPS, H, 3 * H], BF16, kind="ExternalInput")
    wh_d = nc.dram_tensor("wh", [STEPS, H, 3 * H], BF16, kind="ExternalInput")
    dw1_d = nc.dram_tensor("dw1", [H, MH], BF16, kind="ExternalInput")
    dw2_d = nc.dram_tensor("dw2", [MH, MH], BF16, kind="ExternalInput")
    dw3_d = nc.dram_tensor("dw3", [MH, 1], BF16, kind="ExternalInput")
    bias_d = nc.dram_tensor("biases", [128, NBIAS], F32, kind="ExternalInput")
    out_d = nc.dram_tensor("out", [1, nnode], F32, kind="ExternalOutput")

    NN2 = 2 * NN

    with TileContext(nc) as tc:
        with (
            tc.tile_pool(name="const", bufs=1) as constp,
            tc.tile_pool(name="sd", bufs=SD_BUFS) as sdp,
            tc.tile_pool(name="dw", bufs=SD_BUFS) as dwp,
            tc.tile_pool(name="xp", bufs=XP_BUFS) as xpp,
            tc.tile_pool(name="eact", bufs=ACT_BUFS) as eactp,
            tc.tile_pool(name="gact", bufs=ACT_BUFS) as gactp,
            tc.tile_pool(name="pb", bufs=PB_BUFS, space=MemorySpace.PSUM) as ppb,
            tc.tile_pool(name="pb2", bufs=PB2_BUFS, space=MemorySpace.PSUM) as ppb2,
        ):
            obs_t = constp.tile([F_IN, nnode], BF16, tag="obs")
            nc.sync.dma_start(obs_t[:], obsT_d[:])
            encw_t = constp.tile([F_IN, H], BF16, tag="encw")
            nc.sync.dma_start(encw_t[:], encw_d[:])
            w1_t = constp.tile([128, STEPS, 2, MH], BF16, tag="w1")
            nc.sync.dma_start(
                w1_t[:], w1_d.rearrange("s (kc p) m -> p s kc m", p=128)
            )
            w2_t = constp.tile([128, STEPS, 2, MH], E4, tag="w2")
            nc.sync.dma_start(
                w2_t[:], w2_d.rearrange("s (kc p) m -> p s kc m", p=128)
            )
            w3_t = constp.tile([128, STEPS, 2, H], E4, tag="w3")
            nc.sync.dma_start(
                w3_t[:], w3_d.rearrange("s (kc p) m -> p s kc m", p=128)
            )
            wi_t = constp.tile([128, STEPS, 3 * H], BF16, tag="wi")
            nc.sync.dma_start(wi_t[:], wi_d.rearrange("s p m -> p s m"))
            wh_t = constp.tile([128, STEPS, 3 * H], BF16, tag="wh")
            nc.sync.dma_start(wh_t[:], wh_d.rearrange("s p m -> p s m"))
            dw1_t = constp.tile([128, MH], BF16, tag="dw1")
            nc.sync.dma_start(dw1_t[:], dw1_d[:])
            dw2_t = constp.tile([128, 2, MH], BF16, tag="dw2")
            nc.sync.dma_start(dw2_t[:], dw2_d.rearrange("(kc p) m -> p kc m", p=128))
            dw3_t = constp.tile([128, 2, 1], BF16, tag="dw3")
            nc.sync.dma_start(dw3_t[:], dw3_d.rearrange("(kc p) m -> p kc m", p=128))
            bias_t = constp.tile([128, NBIAS], F32, tag="bias")
            nc.sync.dma_start(bias_t[:], bias_d[:])

            def bcol(c):
                return bias_t[:, c : c + 1]

            tot = cfg["totblk"]
            pairs = [tuple(range(kk, min(kk + 2, tot))) for kk in range(0, tot, 2)]

            def geom(k):
                full = k < nblk
                nn = NN if full else tnn
                ne = NE if full else tne
                ecs = [(0, 128), (128, 128), (256, 94)] if full else [(0, tne)]
                return nn, ne, ecs

            class Ctx:
                pass

            def ph_load(cx):
                cx.sds, cx.dws = [], []
                for bi, k in enumerate(cx.pr):
                    nn, ne, ecs = cx.geos[bi]
                    sd = sdp.tile([NN, 2, NEP], E4, tag="sd", name="sd")
                    if k < nblk:
                        nc.sync.dma_start(sd[:, :, :], sdt_d[k])
                    else:
                        nc.sync.dma_start(sd[:nn, 0, :ne], sdt_d[k, :nn, 0, :ne])
                        nc.sync.dma_start(
                            sd[:nn, 1, :ne], sdt_d[k, :nn, 1, :ne]
                        )
                    dwti = dwp.tile([128, 3, NN], BF16, tag="dw", name="dw")
                    nch = len(ecs)
                    nc.sync.dma_start(
                        dwti[:, :nch, :nn],
                        dwt_d[k, :nch, :, :nn].rearrange("c p f -> p c f"),
                    )
                    cx.sds.append(sd)
                    cx.dws.append(dwti)

            def ph_enc(cx):
                penc = ppb.tile([128, 512], F32, tag="pb", name="penc")
                nc.tensor.matmul(
                    penc[:, : cx.npair], encw_t[:, :], obs_t[:, cx.pcols],
                    start=True, stop=True,
                )
                cx.xcur = xpp.tile([128, NN2], BF16, tag="xp", name="x0")
                nc.scalar.activation(
                    cx.xcur[:, : cx.npair], penc[:, : cx.npair], AF.Tanh,
                    bias=bcol(COL_ENC),
                )

            def ph_P(cx, s):
                # P = x @ W1-halves in node space (row layout), then to SBUF
                # as e4m3 for the fp8 DoubleRow gather.
                cx.psb = eactp.tile(
                    [128, 2, 2, 2, 128], E4, tag="psb", name="psb"
                )  # [part, blk, h, mc, feat]
                for bi in range(len(cx.pr)):
                    nn = cx.geos[bi][0]
                    c0 = NN * bi
                    pq = ppb.tile([128, 512], F32, tag="pb", name="pq")
                    for h in range(2):
                        for mc in range(2):
                            o = 256 * h + 128 * mc
                            nc.tensor.matmul(
                                pq[:nn, o : o + 128],
                                cx.xcur[:, c0 : c0 + nn],
                                w1_t[:, s, h, mc * 128 : mc * 128 + 128],
                                start=True, stop=True,
                            )
                    nc.vector.tensor_copy(cx.psb[:nn, bi, :, :, :], pq[:nn, :])

            def ph_m1(cx, s):
                cx.m1sb = eactp.tile(
                    [128, 2, 2, NEP], E4, tag="m1", name="m1sb"
                )  # [part, mc, blk, edge]
                for mc in range(2):
                    pm = ppb2.tile([128, 1024], F32, tag="pb2", name="pm")
                    for bi in range(len(cx.pr)):
                        nn, ne, _ = cx.geos[bi]
                        o = 512 * bi
                        # fp8 DoubleRow: contracts [A;B] (2*nn rows) in one
                        # pass; groups = dst/src halves of the one-hot.
                        nc.tensor.matmul(
                            pm[:, o : o + ne],
                            cx.psb[:nn, bi, :, mc, :],
                            cx.sds[bi][:nn, :, :ne],
                            start=True, stop=True,
                            perf_mode=DR,
                        )
                    if cx.uni:
                        ne = cx.geos[0][1]
                        nc.scalar.activation(
                            cx.m1sb[:, mc, :, :ne],
                            pm.rearrange("p (b f) -> p b f", b=2)[:, :, :ne],
                            AF.Tanh, bias=bcol(COL_B1(s, mc)),
                        )
                    else:
                        for bi in range(len(cx.pr)):
                            ne = cx.geos[bi][1]
                            nc.scalar.activation(
                                cx.m1sb[:, mc, bi, :ne],
                                pm[:, 512 * bi : 512 * bi + ne],
                                AF.Tanh, bias=bcol(COL_B1(s, mc)),
                            )

            def ph_m2(cx, s):
                cx.m2sb = eactp.tile([128, 2, 2, NEP], E4, tag="m2", name="m2sb")
                for mc in range(2):
                    pm = ppb2.tile([128, 1024], F32, tag="pb2", name="pm2")
                    for bi in range(len(cx.pr)):
                        nn, ne, _ = cx.geos[bi]
                        o = 512 * bi
                        nc.tensor.matmul(
                            pm[:, o : o + ne],
                            w2_t[:, s, :, mc * 128 : mc * 128 + 128],
                            cx.m1sb[:, :, bi, :ne],
                            start=True, stop=True,
                            perf_mode=DR,
                        )
                    if cx.uni:
                        ne = cx.geos[0][1]
                        nc.scalar.activation(
                            cx.m2sb[:, mc, :, :ne],
                            pm.rearrange("p (b f) -> p b f", b=2)[:, :, :ne],
                            AF.Tanh, bias=bcol(COL_B2(s, mc)),
                        )
                    else:
                        for bi in range(len(cx.pr)):
                            ne = cx.geos[bi][1]
                            nc.scalar.activation(
                                cx.m2sb[:, mc, bi, :ne],
                                pm[:, 512 * bi : 512 * bi + ne],
                                AF.Tanh, bias=bcol(COL_B2(s, mc)),
                            )

            def ph_w3(cx, s):
                # W3 row-form + scatter into one pair tile:
                # per block bi at 512*bi: m3 chunks [0:384], aggr [384:384+nn]
                cx.m3sb = eactp.tile([128, 2, 3, 128], BF16, tag="m3r", name="m3sb")
                cx.aggp = gactp.tile([128, NN2], BF16, tag="aggr", name="aggp")
                aoff = 0
                for bi in range(len(cx.pr)):
                    nn, _, ecs = cx.geos[bi]
                    pg3 = ppb.tile([128, 512], F32, tag="pb", name="pg3")
                    for ci, (e0, el) in enumerate(ecs):
                        nc.tensor.matmul(
                            pg3[:el, 128 * ci : 128 * ci + 128],
                            cx.m2sb[:, :, bi, e0 : e0 + el],
                            w3_t[:, s, :, :],
                            start=True, stop=True,
                            perf_mode=DR,
                        )
                    nch = len(ecs)
                    nfull = sum(1 for _, el in ecs if el == 128)
                    if nfull:
                        nc.vector.tensor_copy(
                            cx.m3sb[:, bi, :nfull, :], pg3[:, : 128 * nfull]
                        )
                    if nfull < nch:
                        el = ecs[nfull][1]
                        nc.vector.tensor_copy(
                            cx.m3sb[:el, bi, nfull, :],
                            pg3[:el, 128 * nfull : 128 * nfull + 128],
                        )
                    for ci, (e0, el) in enumerate(ecs):
                        nc.tensor.matmul(
                            pg3[:, 384 : 384 + nn],
                            cx.m3sb[:el, bi, ci, :],
                            cx.dws[bi][:el, ci, :nn],
                            start=(ci == 0), stop=(ci == len(ecs) - 1),
                        )
                    nc.vector.tensor_copy(
                        cx.aggp[:, aoff : aoff + nn], pg3[:, 384 : 384 + nn]
                    )
                    aoff += nn

            def ph_gru(cx, s):
                npair = cx.npair
                pgr = ppb.tile([128, 512], F32, tag="pb", name="pgr")
                pgn = ppb.tile([128, 512], F32, tag="pb", name="pgn")
                # pgr: rz0@0, rz1@npair; pgn: gin@0, ghn@npair
                for g, off in ((0, 0), (1, npair)):
                    nc.tensor.matmul(
                        pgr[:, off : off + npair],
                        wi_t[:, s, g * 128 : g * 128 + 128],
                        cx.aggp[:, :npair],
                        start=True, stop=False,
                    )
                    nc.tensor.matmul(
                        pgr[:, off : off + npair],
                        wh_t[:, s, g * 128 : g * 128 + 128],
                        cx.xcur[:, :npair],
                        start=False, stop=True,
                    )
                nc.tensor.matmul(
                    pgn[:, :npair],
                    wi_t[:, s, 256:384],
                    cx.aggp[:, :npair],
                    start=True, stop=True,
                )
                nc.tensor.matmul(
                    pgn[:, npair : 2 * npair],
                    wh_t[:, s, 256:384],
                    cx.xcur[:, :npair],
                    start=True, stop=True,
                )
                thr = gactp.tile([128, NN2], BF16, tag="thr", name="thr")
                cx.thz = gactp.tile([128, NN2], BF16, tag="thz", name="thz")
                nc.scalar.activation(
                    thr[:, :npair], pgr[:, :npair], AF.Tanh,
                    bias=bcol(COL_BRZ(s, 0)), scale=0.5,
                )
                nc.scalar.activation(
                    cx.thz[:, :npair], pgr[:, npair : 2 * npair], AF.Tanh,
                    bias=bcol(COL_BRZ(s, 1)), scale=0.5,
                )
                hnp = gactp.tile([128, NN2], BF16, tag="hnp", name="hnp")
                nc.vector.tensor_scalar(
                    hnp[:, :npair],
                    pgn[:, npair : 2 * npair],
                    bcol(COL_BHN(s)), 0.5,
                    op0=ALU.add, op1=ALU.mult,
                )
                rhn = gactp.tile([128, NN2], BF16, tag="rhn", name="rhn")
                nc.vector.scalar_tensor_tensor(
                    rhn[:, :npair], thr[:, :npair], 1.0, hnp[:, :npair],
                    op0=ALU.add, op1=ALU.mult,
                )
                cx.tn = gactp.tile([128, NN2], BF16, tag="tn", name="tn")
                nc.vector.scalar_tensor_tensor(
                    cx.tn[:, :npair], pgn[:, :npair],
                    bcol(COL_BIN(s)), rhn[:, :npair],
                    op0=ALU.add, op1=ALU.add,
                )

            def ph_xupd(cx, s):
                npair = cx.npair
                ngate = gactp.tile([128, NN2], BF16, tag="ng", name="ng")
                nc.scalar.activation(ngate[:, :npair], cx.tn[:, :npair], AF.Tanh)
                # x' = n + z*(x-n), z = 0.5*(1+th_z)
                d_ = gactp.tile([128, NN2], BF16, tag="d", name="d_")
                nc.gpsimd.tensor_sub(
                    d_[:, :npair], cx.xcur[:, :npair], ngate[:, :npair]
                )
                w_ = gactp.tile([128, NN2], BF16, tag="w", name="w_")
                nc.vector.scalar_tensor_tensor(
                    w_[:, :npair], cx.thz[:, :npair], 1.0, d_[:, :npair],
                    op0=ALU.add, op1=ALU.mult,
                )
                xnxt = xpp.tile([128, NN2], BF16, tag="xp", name="xn")
                nc.vector.scalar_tensor_tensor(
                    xnxt[:, :npair], w_[:, :npair], 0.5, ngate[:, :npair],
                    op0=ALU.mult, op1=ALU.add,
                )
                cx.xcur = xnxt

            def ph_dec1(cx):
                npair = cx.npair
                pd1 = ppb.tile([128, 512], F32, tag="pb", name="pd1")
                cx.d1sb = gactp.tile([128, 2, NN2], BF16, tag="d1", name="d1sb")
                for mc in range(2):
                    nc.tensor.matmul(
                        pd1[:, 252 * mc : 252 * mc + npair],
                        dw1_t[:, mc * 128 : mc * 128 + 128],
                        cx.xcur[:, :npair],
                        start=True, stop=True,
                    )
                    nc.scalar.activation(
                        cx.d1sb[:, mc, :npair],
                        pd1[:, 252 * mc : 252 * mc + npair],
                        AF.Tanh, bias=bcol(COL_DB1(mc)),
                    )

            def ph_dec2(cx):
                npair = cx.npair
                pd2 = ppb.tile([128, 512], F32, tag="pb", name="pd2")
                d2sb = gactp.tile([128, 2, NN2], BF16, tag="d2", name="d2sb")
                for mc in range(2):
                    for kc in range(2):
                        nc.tensor.matmul(
                            pd2[:, 252 * mc : 252 * mc + npair],
                            dw2_t[:, kc, mc * 128 : mc * 128 + 128],
                            cx.d1sb[:, kc, :npair],
                            start=(kc == 0), stop=(kc == 1),
                        )
                    nc.scalar.activation(
                        d2sb[:, mc, :npair],
                        pd2[:, 252 * mc : 252 * mc + npair],
                        AF.Tanh, bias=bcol(COL_DB2(mc)),
                    )
                pd3 = ppb.tile([128, 512], F32, tag="pb", name="pd3")
                for kc in range(2):
                    nc.tensor.matmul(
                        pd3[:1, :npair], dw3_t[:, kc, :], d2sb[:, kc, :npair],
                        start=(kc == 0), stop=(kc == 1),
                    )
                outp = gactp.tile([1, NN2], F32, tag="outp", name="outp")
                nc.scalar.activation(
                    outp[:, :npair], pd3[:1, :npair], AF.Identity,
                    bias=bias_t[0:1, COL_DB3 : COL_DB3 + 1],
                )
                nc.sync.dma_start(out_d[:, cx.pcols], outp[:1, :npair])

            for _rep in range(repeat):
                allpairs = list(pairs)
                waves = [
                    allpairs[i : i + WAVE] for i in range(0, len(allpairs), WAVE)
                ]
                for wv in waves:
                    cxs = []
                    for pr in wv:
                        cx = Ctx()
                        cx.pr = pr
                        cx.geos = [geom(k) for k in pr]
                        cx.uni = len(pr) == 2 and cx.geos[0] == cx.geos[1]
                        cx.col0 = NN * pr[0]
                        cx.npair = sum(g[0] for g in cx.geos)
                        cx.pcols = slice(cx.col0, cx.col0 + cx.npair)
                        cxs.append(cx)
                    for cx in cxs:
                        ph_load(cx)
                    for cx in cxs:
                        ph_enc(cx)
                    for s in range(STEPS):
                        for ph in (ph_P, ph_m1, ph_m2, ph_w3, ph_gru, ph_xupd):
                            for cx in cxs:
                                ph(cx, s)
                    for cx in cxs:
                        ph_dec1(cx)
                    for cx in cxs:
                        ph_dec2(cx)

    nc.compile()
    _NC_CACHE[key] = nc
    return nc


def bench_hw(inputs, repeats=(1, 3), n_iter=14):
    """Differential HW timing: wall-clock difference between NEFFs that run
    the message-passing phase R times (dispatch overhead cancels)."""
    from test import bench  # local harness helper

    in_maps = preprocess(inputs, CFG_FULL)
    res = {}
    for r in repeats:
        nc = build_nc(CFG_FULL, repeat=r)
        times, _ = bench(nc, in_maps, n_iter=n_iter)
        ts = np.sort(times)[: max(3, n_iter // 2)]
        res[r] = ts.mean()
        print(f"repeat={r}: min {times.min()*1e3:.3f} ms  "
              f"low-half-mean {ts.mean()*1e3:.3f} ms")
    rs = sorted(res)
    phase = (res[rs[-1]] - res[rs[0]]) / (rs[-1] - rs[0])
    print(f"block-phase time ≈ {phase*1e3:.3f} ms")
    return phase, res


def preprocess(inputs, cfg):
    bg, nblk, tailg, totblk, nnode = (
        cfg["bg"], cfg["nblk"], cfg["tailg"], cfg["totblk"], cfg["nnode"],
    )
    b = bg * NCORES
    obs = np.asarray(inputs["obs"], np.float32)
    edges = np.asarray(inputs["edges"], np.int64)

    # one-hot incidence per graph
    src = edges[:, 0, :]
    dst = edges[:, 1, :]
    loops = np.broadcast_to(np.arange(N, dtype=np.int64), (b, N))
    src_all = np.concatenate([src, loops], 1)  # [b, 25]
    dst_all = np.concatenate([dst, loops], 1)
    nod = np.arange(N, dtype=np.int64)
    Sg = (src_all[:, None, :] == nod[None, :, None]).astype(np.float32)  # [b,9,25]
    Dg = (dst_all[:, None, :] == nod[None, :, None]).astype(np.float32)  # [b,9,25]
    deg = Dg.sum(2)  # [b, 9] >= 1
    Dw = Dg.transpose(0, 2, 1) / deg[:, None, :]  # [b, 25, 9]

    SDt = np.zeros((NCORES, totblk, NN, 2 * NE), NPE4)
    DWf = np.zeros((NCORES, totblk, 384, NN), np.float32)
    Sg_ = Sg.reshape(NCORES, bg, N, EPG)
    Dg_ = Dg.reshape(NCORES, bg, N, EPG)
    Dw_ = Dw.reshape(NCORES, bg, EPG, N)
    nmain = nblk * GPB
    Sm = Sg_[:, :nmain].reshape(NCORES, nblk, GPB, N, EPG)
    Dm = Dg_[:, :nmain].reshape(NCORES, nblk, GPB, N, EPG)
    Wm = Dw_[:, :nmain].reshape(NCORES, nblk, GPB, EPG, N)
    for i in range(GPB):
        r = slice(N * i, N * i + N)
        c = slice(EPG * i, EPG * i + EPG)
        SDt[:, :nblk, r, c] = Dm[:, :, i]  # dst-gather one-hot
        SDt[:, :nblk, r, NE + EPG * i : NE + EPG * i + EPG] = Sm[:, :, i]
        DWf[:, :nblk, c, r] = Wm[:, :, i]
    for i in range(tailg):
        g = nmain + i
        r = slice(N * i, N * i + N)
        c = slice(EPG * i, EPG * i + EPG)
        SDt[:, nblk, r, c] = Dg_[:, g]
        SDt[:, nblk, r, NE + EPG * i : NE + EPG * i + EPG] = Sg_[:, g]
        DWf[:, nblk, c, r] = Dw_[:, g]
    DWt = DWf.reshape(NCORES, totblk, 3, 128, NN).astype(NPBF16)

    obsT = (
        obs.reshape(b, N, F_IN)
        .reshape(NCORES, nnode, F_IN)
        .transpose(0, 2, 1)
        .astype(NPBF16)
    )  # [8, 15, nnode]

    f32 = lambda x: np.asarray(x, np.float32)
    bf = lambda x: np.ascontiguousarray(f32(x)).astype(NPBF16)
    e4 = lambda x: np.ascontiguousarray(f32(x)).astype(NPE4)

    biases = np.zeros((128, NBIAS), np.float32)
    biases[:, COL_ENC] = f32(inputs["enc_b"])
    gru_bi = f32(inputs["gru_bi"])
    gru_bh = f32(inputs["gru_bh"])
    msg_b3 = f32(inputs["msg_b3"])
    gru_Wi = f32(inputs["gru_Wi"])
    for s in range(STEPS):
        b1 = f32(inputs["msg_b1"][s])
        b2 = f32(inputs["msg_b2"][s])
        for c in range(2):
            biases[:, COL_B1(s, c)] = b1[128 * c : 128 * (c + 1)]
            biases[:, COL_B2(s, c)] = b2[128 * c : 128 * (c + 1)]
        bip = gru_bi[s] + msg_b3[s] @ gru_Wi[s]  # fold msg_b3 into GRU input bias
        for g in range(2):
            biases[:, COL_BRZ(s, g)] = 0.5 * (
                bip[128 * g : 128 * (g + 1)] + gru_bh[s][128 * g : 128 * (g + 1)]
            )
        biases[:, COL_BHN(s)] = gru_bh[s][256:384]
        biases[:, COL_BIN(s)] = bip[256:384]
    db1 = f32(inputs["dec_b1"])
    db2 = f32(inputs["dec_b2"])
    for c in range(2):
        biases[:, COL_DB1(c)] = db1[128 * c : 128 * (c + 1)]
        biases[:, COL_DB2(c)] = db2[128 * c : 128 * (c + 1)]
    biases[0, COL_DB3] = float(f32(inputs["dec_b3"])[0])

    shared = dict(
        encw=bf(inputs["enc_W"]),
        w1=bf(inputs["msg_W1"]),
        w2=e4(inputs["msg_W2"]),
        w3=e4(inputs["msg_W3"]),
        wi=bf(inputs["gru_Wi"]),
        wh=bf(inputs["gru_Wh"]),
        dw1=bf(inputs["dec_W1"]),
        dw2=bf(inputs["dec_W2"]),
        dw3=bf(inputs["dec_W3"]),
        biases=biases,
    )
    SDp = np.zeros((NCORES, totblk, NN, 2, NEP), NPE4)
    SDv = SDt.reshape(NCORES, totblk, NN, 2, NE)
    SDp[:, :, :, :, :NE] = SDv
    in_maps = []
    for c in range(NCORES):
        m = dict(shared)
        m["obsT"] = np.ascontiguousarray(obsT[c])
        m["sdt"] = np.ascontiguousarray(SDp[c])
        m["dwt"] = np.ascontiguousarray(DWt[c])
        in_maps.append(m)
    return in_maps


LAST_EXEC_NS = None
TRACE = False


def _run(inputs, cfg):
    global LAST_EXEC_NS
    nc = build_nc(cfg)
    in_maps = preprocess(inputs, cfg)
    res = run_bass_kernel_spmd(
        nc, in_maps, core_ids=list(range(NCORES)), trace=TRACE
    )
    LAST_EXEC_NS = res.exec_time_ns
    bg = cfg["bg"]
    outs = [np.asarray(res.results[c]["out"], np.float32).reshape(bg, N) for c in range(NCORES)]
    full = np.concatenate(outs, 0)  # [B, 9]
    return np.ascontiguousarray(full[:, :8])


def kernel(**inputs) -> np.ndarray:
    return _run(inputs, CFG_FULL)



# revision 52
# speedup vs baseline: 2.7186x; 1.1703x over previous
"""Trainium2 Bass kernel for nn_MessagePassingGNN (B=8192 graphs, N=9 nodes,
16 edges + 9 self-loops per graph, 4 message-passing steps + GRU, decoder).

Strategy:
  - Data-parallel over batch: each of 8 cores gets 1024 graphs.
  - Within a core, graphs are packed into blocks of 14 (126 nodes, 350 edges)
    plus one tail block of 2 graphs, processed as 37 block-pairs. Gather
    (x[dst], x[src]) and scatter (mean-aggregation) are matmuls against
    host-precomputed one-hot incidence matrices, so the whole step pipeline
    lives on the TensorEngine.
  - No transposes anywhere: W1 is applied FIRST in node space (P = X @ W1,
    with x kept transposed [feat, nodes] so x itself is the stationary
    operand), and the gather then accumulates P halves in edge space. W3 is
    emitted in row form (stationary = m2^T slices) so the scatter gets its
    edge-major operand for free.
  - Node-space matmuls (encoder, P, GRU, decoder, scatter) are bf16; the
    edge-space pipeline (gather, m2, m3) runs fp8-e4m3 with DoubleRow perf
    mode: one matmul contracts 2x128 rows (dst+src halves for the gather,
    both 128-chunks of the 256-wide contraction for m2/m3), halving
    TensorE cycles there. One-hot incidence entries are exact in fp8.
    Sigmoid is rewritten via tanh (z = 0.5*(1+tanh(g/2))) so ScalarE needs
    a single LUT table set.
  - deg-normalization is folded into the scatter one-hot; msg_b3 is folded
    into the GRU input bias (host-side), so no partition-broadcast is needed.
  - Encoder/decoder are per-pair (no serial phases); WAVE=6 pairs are emitted
    phase-interleaved so each engine's in-order stream has independent work
    to fill cross-engine dependency stalls; PSUM is rotated as 4x 1-bank +
    2x 2-bank pool slots.
"""

import numpy as np

try:
    import concourse.bass as bass  # noqa: F401
except Exception:  # pragma: no cover
    import sys

    sys.path.insert(0, "/opt/trn_rl_repo")

import ml_dtypes
import concourse.bass as bass
import concourse.bacc as bacc
import concourse.mybir as mybir
from concourse.bass import MemorySpace
from concourse.bass_utils import run_bass_kernel_spmd
from concourse.tile import TileContext

BF16 = mybir.dt.bfloat16
F32 = mybir.dt.float32
E4 = mybir.dt.float8e4
NPBF16 = ml_dtypes.bfloat16
NPE4 = ml_dtypes.float8_e4m3
AF = mybir.ActivationFunctionType
ALU = mybir.AluOpType
DR = mybir.MatmulPerfMode.DoubleRow

N, F_IN, H, MH, STEPS = 9, 15, 128, 256, 4
E_PER = 16
EPG = E_PER + N  # 25 edges per graph incl self-loops
NCORES = 8
GPB = 14  # graphs per full block
NN = GPB * N  # 126 nodes per full block
NE = GPB * EPG  # 350 edges per full block
NEP = 352  # NE padded so fp8 DoubleRow group strides are 16B-aligned

# bias-pack column map
COL_ENC = 0
COL_B1 = lambda s, c: 1 + 2 * s + c
COL_B2 = lambda s, c: 9 + 2 * s + c
COL_BRZ = lambda s, g: 17 + 2 * s + g  # 0.5*(bi'+bh)[g*128:+128]
COL_BHN = lambda s: 25 + s  # bh[256:384]
COL_BIN = lambda s: 29 + s  # bi'[256:384]
COL_DB1 = lambda c: 33 + c
COL_DB2 = lambda c: 35 + c
COL_DB3 = 37
NBIAS = 38


def _derive(bg):
    nblk = bg // GPB
    tailg = bg - nblk * GPB
    totblk = nblk + (1 if tailg else 0)
    nnode = bg * N
    return dict(bg=bg, nblk=nblk, tailg=tailg, totblk=totblk, nnode=nnode)


CFG_FULL = _derive(1024)

_NC_CACHE = {}
PHASE_HOOK = None  # profiling-only: called as PHASE_HOOK(nc, label) at phase entry


GSKEW = 2      # pairs per skewed pipeline group
SD_BUFS = 26
XP_BUFS = 18
ACT_BUFS = 10
PB_BUFS = 4
PB2_BUFS = 2


def build_nc(cfg, repeat=1):
    key = (cfg["bg"], repeat, GSKEW, SD_BUFS, XP_BUFS, ACT_BUFS, PB_BUFS, PB2_BUFS)
    if key in _NC_CACHE:
        return _NC_CACHE[key]
    nblk, tailg, totblk, nnode = (
        cfg["nblk"],
        cfg["tailg"],
        cfg["totblk"],
        cfg["nnode"],
    )
    tnn, tne = tailg * N, tailg * EPG

    nc = bacc.Bacc("TRN2", target_bir_lowering=False, debug=False, num_devices=NCORES)

    obsT_d = nc.dram_tensor("obsT", [F_IN, nnode], BF16, kind="ExternalInput")
    sdt_d = nc.dram_tensor("sdt", [totblk, NN, 2, NEP], E4, kind="ExternalInput")
    dwt_d = nc.dram_tensor("dwt", [totblk, 3, 128, NN], BF16, kind="ExternalInput")
    encw_d = nc.dram_tensor("encw", [F_IN, H], BF16, kind="ExternalInput")
    w1_d = nc.dram_tensor("w1", [STEPS, 2 * H, MH], BF16, kind="ExternalInput")
    w2_d = nc.dram_tensor("w2", [STEPS, MH, MH], E4, kind="ExternalInput")
    w3_d = nc.dram_tensor("w3", [STEPS, MH, H], E4, kind="ExternalInput")
    wi_d = nc.dram_tensor("wi", [STEPS, H, 3 * H], BF16, kind="ExternalInput")
    wh_d = nc.dram_tensor("wh", [STEPS, H, 3 * H], BF16, kind="ExternalInput")
    dw1_d = nc.dram_tensor("dw1", [H, MH], BF16, kind="ExternalInput")
    dw2_d = nc.dram_tensor("dw2", [MH, MH], BF16, kind="ExternalInput")
    dw3_d = nc.dram_tensor("dw3", [MH, 1], BF16, kind="ExternalInput")
    bias_d = nc.dram_tensor("biases", [128, NBIAS], F32, kind="ExternalInput")
    # row-form biases for PE-side bias accumulation (ones-row matmul):
    # rows 2s+g = gru (bi'+bh) gate chunks, 8+mc = dec_b1, 10+mc = dec_b2
    brow_d = nc.dram_tensor("brow", [1, 16, 128], BF16, kind="ExternalInput")
    out_d = nc.dram_tensor("out", [1, nnode], F32, kind="ExternalOutput")

    NN2 = 2 * NN

    with TileContext(nc) as tc:
        with (
            tc.tile_pool(name="const", bufs=1) as constp,
            tc.tile_pool(name="sd", bufs=SD_BUFS) as sdp,
            tc.tile_pool(name="dw", bufs=SD_BUFS) as dwp,
            tc.tile_pool(name="xp", bufs=XP_BUFS) as xpp,
            tc.tile_pool(name="eact", bufs=ACT_BUFS) as eactp,
            tc.tile_pool(name="gact", bufs=ACT_BUFS) as gactp,
            tc.tile_pool(name="tact", bufs=4) as tactp,
            tc.tile_pool(name="pb", bufs=PB_BUFS, space=MemorySpace.PSUM) as ppb,
            tc.tile_pool(name="pb2", bufs=PB2_BUFS, space=MemorySpace.PSUM) as ppb2,
        ):
            obs_t = constp.tile([F_IN, nnode], BF16, tag="obs")
            nc.sync.dma_start(obs_t[:], obsT_d[:])
            encw_t = constp.tile([F_IN, H], BF16, tag="encw")
            nc.sync.dma_start(encw_t[:], encw_d[:])
            w1_t = constp.tile([128, STEPS, 2, MH], BF16, tag="w1")
            nc.sync.dma_start(
                w1_t[:], w1_d.rearrange("s (kc p) m -> p s kc m", p=128)
            )
            w2_t = constp.tile([128, STEPS, 2, MH], E4, tag="w2")
            nc.sync.dma_start(
                w2_t[:], w2_d.rearrange("s (kc p) m -> p s kc m", p=128)
            )
            w3_t = constp.tile([128, STEPS, 2, H], E4, tag="w3")
            nc.sync.dma_start(
                w3_t[:], w3_d.rearrange("s (kc p) m -> p s kc m", p=128)
            )
            wi_t = constp.tile([128, STEPS, 3 * H], BF16, tag="wi")
            nc.sync.dma_start(wi_t[:], wi_d.rearrange("s p m -> p s m"))
            wh_t = constp.tile([128, STEPS, 3 * H], BF16, tag="wh")
            nc.sync.dma_start(wh_t[:], wh_d.rearrange("s p m -> p s m"))
            dw1_t = constp.tile([128, MH], BF16, tag="dw1")
            nc.sync.dma_start(dw1_t[:], dw1_d[:])
            dw2_t = constp.tile([128, 2, MH], BF16, tag="dw2")
            nc.sync.dma_start(dw2_t[:], dw2_d.rearrange("(kc p) m -> p kc m", p=128))
            dw3_t = constp.tile([128, 2, 1], BF16, tag="dw3")
            nc.sync.dma_start(dw3_t[:], dw3_d.rearrange("(kc p) m -> p kc m", p=128))
            bias_t = constp.tile([128, NBIAS], F32, tag="bias")
            nc.sync.dma_start(bias_t[:], bias_d[:])
            brow_t = constp.tile([1, 16, 128], BF16, tag="brow")
            nc.sync.dma_start(brow_t[:], brow_d[:])
            ones_t = constp.tile([1, NN2], BF16, tag="ones")
            nc.gpsimd.memset(ones_t[:], 1.0)

            def bcol(c):
                return bias_t[:, c : c + 1]

            tot = cfg["totblk"]
            pairs = [tuple(range(kk, min(kk + 2, tot))) for kk in range(0, tot, 2)]

            def geom(k):
                full = k < nblk
                nn = NN if full else tnn
                ne = NE if full else tne
                ecs = [(0, 128), (128, 128), (256, 94)] if full else [(0, tne)]
                return nn, ne, ecs

            class Ctx:
                pass

            def ph_load(cx):
                if PHASE_HOOK:
                    PHASE_HOOK(nc, "ph_load")
                cx.sds, cx.dws = [], []
                for bi, k in enumerate(cx.pr):
                    nn, ne, ecs = cx.geos[bi]
                    sd = sdp.tile([NN, 2, NEP], E4, tag="sd", name="sd")
                    if k < nblk:
                        nc.sync.dma_start(sd[:, :, :], sdt_d[k])
                    else:
                        nc.sync.dma_start(sd[:nn, 0, :ne], sdt_d[k, :nn, 0, :ne])
                        nc.sync.dma_start(
                            sd[:nn, 1, :ne], sdt_d[k, :nn, 1, :ne]
                        )
                    dwti = dwp.tile([128, 3, NN], BF16, tag="dw", name="dw")
                    nch = len(ecs)
                    nc.sync.dma_start(
                        dwti[:, :nch, :nn],
                        dwt_d[k, :nch, :, :nn].rearrange("c p f -> p c f"),
                    )
                    cx.sds.append(sd)
                    cx.dws.append(dwti)

            def ph_enc(cx):
                if PHASE_HOOK:
                    PHASE_HOOK(nc, "ph_enc")
                penc = ppb.tile([128, 512], F32, tag="pb", name="penc")
                nc.tensor.matmul(
                    penc[:, : cx.npair], encw_t[:, :], obs_t[:, cx.pcols],
                    start=True, stop=True,
                )
                cx.xcur = xpp.tile([128, NN2], BF16, tag="xp", name="x0")
                nc.scalar.activation(
                    cx.xcur[:, : cx.npair], penc[:, : cx.npair], AF.Tanh,
                    bias=bcol(COL_ENC),
                )

            def ph_P(cx, s):
                if PHASE_HOOK:
                    PHASE_HOOK(nc, "ph_P")
                # P = x @ W1-halves in node space (row layout), then to SBUF
                # as e4m3 for the fp8 DoubleRow gather.
                cx.psb = eactp.tile(
                    [128, 2, 2, 2, 128], E4, tag="psb", name="psb"
                )  # [part, blk, h, mc, feat]
                for bi in range(len(cx.pr)):
                    nn = cx.geos[bi][0]
                    c0 = NN * bi
                    pq = ppb.tile([128, 512], F32, tag="pb", name="pq")
                    for h in range(2):
                        for mc in range(2):
                            o = 256 * h + 128 * mc
                            nc.tensor.matmul(
                                pq[:nn, o : o + 128],
                                cx.xcur[:, c0 : c0 + nn],
                                w1_t[:, s, h, mc * 128 : mc * 128 + 128],
                                start=True, stop=True,
                            )
                    nc.vector.tensor_copy(cx.psb[:nn, bi, :, :, :], pq[:nn, :])

            def ph_m1(cx, s):
                if PHASE_HOOK:
                    PHASE_HOOK(nc, "ph_m1")
                cx.m1sb = eactp.tile(
                    [128, 2, 2, NEP], E4, tag="m1", name="m1sb"
                )  # [part, mc, blk, edge]
                for mc in range(2):
                    pm = ppb2.tile([128, 1024], F32, tag="pb2", name="pm")
                    for bi in range(len(cx.pr)):
                        nn, ne, _ = cx.geos[bi]
                        o = 512 * bi
                        # fp8 DoubleRow: contracts [A;B] (2*nn rows) in one
                        # pass; groups = dst/src halves of the one-hot.
                        nc.tensor.matmul(
                            pm[:, o : o + ne],
                            cx.psb[:nn, bi, :, mc, :],
                            cx.sds[bi][:nn, :, :ne],
                            start=True, stop=True,
                            perf_mode=DR,
                        )
                    if cx.uni:
                        ne = cx.geos[0][1]
                        nc.scalar.activation(
                            cx.m1sb[:, mc, :, :ne],
                            pm.rearrange("p (b f) -> p b f", b=2)[:, :, :ne],
                            AF.Tanh, bias=bcol(COL_B1(s, mc)),
                        )
                    else:
                        for bi in range(len(cx.pr)):
                            ne = cx.geos[bi][1]
                            nc.scalar.activation(
                                cx.m1sb[:, mc, bi, :ne],
                                pm[:, 512 * bi : 512 * bi + ne],
                                AF.Tanh, bias=bcol(COL_B1(s, mc)),
                            )

            def ph_m2(cx, s):
                if PHASE_HOOK:
                    PHASE_HOOK(nc, "ph_m2")
                cx.m2sb = eactp.tile([128, 2, 2, NEP], E4, tag="m2", name="m2sb")
                for mc in range(2):
                    pm = ppb2.tile([128, 1024], F32, tag="pb2", name="pm2")
                    for bi in range(len(cx.pr)):
                        nn, ne, _ = cx.geos[bi]
                        o = 512 * bi
                        nc.tensor.matmul(
                            pm[:, o : o + ne],
                            w2_t[:, s, :, mc * 128 : mc * 128 + 128],
                            cx.m1sb[:, :, bi, :ne],
                            start=True, stop=True,
                            perf_mode=DR,
                        )
                    if cx.uni:
                        ne = cx.geos[0][1]
                        nc.scalar.activation(
                            cx.m2sb[:, mc, :, :ne],
                            pm.rearrange("p (b f) -> p b f", b=2)[:, :, :ne],
                            AF.Tanh, bias=bcol(COL_B2(s, mc)),
                        )
                    else:
                        for bi in range(len(cx.pr)):
                            ne = cx.geos[bi][1]
                            nc.scalar.activation(
                                cx.m2sb[:, mc, bi, :ne],
                                pm[:, 512 * bi : 512 * bi + ne],
                                AF.Tanh, bias=bcol(COL_B2(s, mc)),
                            )

            def ph_w3(cx, s):
                if PHASE_HOOK:
                    PHASE_HOOK(nc, "ph_w3")
                # W3 row-form + scatter into one pair tile:
                # per block bi at 512*bi: m3 chunks [0:384], aggr [384:384+nn]
                cx.m3sb = eactp.tile([128, 2, 3, 128], BF16, tag="m3r", name="m3sb")
                cx.aggp = gactp.tile([128, NN2], BF16, tag="aggr", name="aggp")
                aoff = 0
                for bi in range(len(cx.pr)):
                    nn, _, ecs = cx.geos[bi]
                    pg3 = ppb.tile([128, 512], F32, tag="pb", name="pg3")
                    for ci, (e0, el) in enumerate(ecs):
                        nc.tensor.matmul(
                            pg3[:el, 128 * ci : 128 * ci + 128],
                            cx.m2sb[:, :, bi, e0 : e0 + el],
                            w3_t[:, s, :, :],
                            start=True, stop=True,
                            perf_mode=DR,
                        )
                    nch = len(ecs)
                    nfull = sum(1 for _, el in ecs if el == 128)
                    if nfull:
                        nc.vector.tensor_copy(
                            cx.m3sb[:, bi, :nfull, :], pg3[:, : 128 * nfull]
                        )
                    if nfull < nch:
                        el = ecs[nfull][1]
                        nc.vector.tensor_copy(
                            cx.m3sb[:el, bi, nfull, :],
                            pg3[:el, 128 * nfull : 128 * nfull + 128],
                        )
                    for ci, (e0, el) in enumerate(ecs):
                        nc.tensor.matmul(
                            pg3[:, 384 : 384 + nn],
                            cx.m3sb[:el, bi, ci, :],
                            cx.dws[bi][:el, ci, :nn],
                            start=(ci == 0), stop=(ci == len(ecs) - 1),
                        )
                    nc.vector.tensor_copy(
                        cx.aggp[:, aoff : aoff + nn], pg3[:, 384 : 384 + nn]
                    )
                    aoff += nn

            def ph_gru(cx, s):
                if PHASE_HOOK:
                    PHASE_HOOK(nc, "ph_gru")
                npair = cx.npair
                pgr = ppb.tile([128, 512], F32, tag="pb", name="pgr")
                pgn = ppb.tile([128, 512], F32, tag="pb", name="pgn")
                # pgr: rz0@0, rz1@npair; pgn: gin@0, ghn@npair
                for g, off in ((0, 0), (1, npair)):
                    nc.tensor.matmul(
                        pgr[:, off : off + npair],
                        wi_t[:, s, g * 128 : g * 128 + 128],
                        cx.aggp[:, :npair],
                        start=True, stop=False,
                    )
                    nc.tensor.matmul(
                        pgr[:, off : off + npair],
                        wh_t[:, s, g * 128 : g * 128 + 128],
                        cx.xcur[:, :npair],
                        start=False, stop=False,
                    )
                    # + (bi'+bh) gate bias via ones-row matmul (PE has slack)
                    nc.tensor.matmul(
                        pgr[:, off : off + npair],
                        brow_t[:, 2 * s + g, :],
                        ones_t[:, :npair],
                        start=False, stop=True,
                    )
                nc.tensor.matmul(
                    pgn[:, :npair],
                    wi_t[:, s, 256:384],
                    cx.aggp[:, :npair],
                    start=True, stop=True,
                )
                nc.tensor.matmul(
                    pgn[:, npair : 2 * npair],
                    wh_t[:, s, 256:384],
                    cx.xcur[:, :npair],
                    start=True, stop=False,
                )
                nc.tensor.matmul(
                    pgn[:, npair : 2 * npair],
                    brow_t[:, 12 + s, :],
                    ones_t[:, :npair],
                    start=False, stop=True,
                )
                thrz = gactp.tile([128, 2, NN2], BF16, tag="thrz", name="thrz")
                nc.scalar.activation(
                    thrz[:, :, :npair],
                    pgr[:, : 2 * npair].rearrange("p (g c) -> p g c", g=2),
                    AF.Tanh, scale=0.5,
                )
                thr = thrz[:, 0, :]
                cx.thz = thrz[:, 1, :]
                rhn = tactp.tile([128, NN2], BF16, tag="rhn", name="rhn")
                nc.vector.scalar_tensor_tensor(
                    rhn[:, :npair], thr[:, :npair], 1.0,
                    pgn[:, npair : 2 * npair],
                    op0=ALU.add, op1=ALU.mult,
                )
                cx.tn = tactp.tile([128, NN2], BF16, tag="tn", name="tn")
                nc.vector.scalar_tensor_tensor(
                    cx.tn[:, :npair], pgn[:, :npair],
                    bcol(COL_BIN(s)), rhn[:, :npair],
                    op0=ALU.add, op1=ALU.add,
                )

            def ph_xupd(cx, s):
                if PHASE_HOOK:
                    PHASE_HOOK(nc, "ph_xupd")
                npair = cx.npair
                ngate = tactp.tile([128, NN2], BF16, tag="ng", name="ng")
                nc.scalar.activation(ngate[:, :npair], cx.tn[:, :npair], AF.Tanh)
                # x' = n + z*(x-n), z = 0.5*(1+th_z)
                d_ = tactp.tile([128, NN2], BF16, tag="d", name="d_")
                nc.gpsimd.tensor_sub(
                    d_[:, :npair], cx.xcur[:, :npair], ngate[:, :npair]
                )
                w_ = tactp.tile([128, NN2], BF16, tag="w", name="w_")
                nc.vector.scalar_tensor_tensor(
                    w_[:, :npair], cx.thz[:, :npair], 1.0, d_[:, :npair],
                    op0=ALU.add, op1=ALU.mult,
                )
                xnxt = xpp.tile([128, NN2], BF16, tag="xp", name="xn")
                nc.vector.scalar_tensor_tensor(
                    xnxt[:, :npair], w_[:, :npair], 0.5, ngate[:, :npair],
                    op0=ALU.mult, op1=ALU.add,
                )
                cx.xcur = xnxt

            def ph_dec1(cx):
                if PHASE_HOOK:
                    PHASE_HOOK(nc, "ph_dec1")
                npair = cx.npair
                pd1 = ppb.tile([128, 512], F32, tag="pb", name="pd1")
                cx.d1sb = gactp.tile([128, 2, NN2], BF16, tag="d1", name="d1sb")
                for mc in range(2):
                    nc.tensor.matmul(
                        pd1[:, npair * mc : npair * mc + npair],
                        dw1_t[:, mc * 128 : mc * 128 + 128],
                        cx.xcur[:, :npair],
                        start=True, stop=False,
                    )
                    nc.tensor.matmul(
                        pd1[:, npair * mc : npair * mc + npair],
                        brow_t[:, 8 + mc, :],
                        ones_t[:, :npair],
                        start=False, stop=True,
                    )
                nc.scalar.activation(
                    cx.d1sb[:, :, :npair],
                    pd1[:, : 2 * npair].rearrange("p (g c) -> p g c", g=2),
                    AF.Tanh,
                )

            def ph_dec2(cx):
                if PHASE_HOOK:
                    PHASE_HOOK(nc, "ph_dec2")
                npair = cx.npair
                pd2 = ppb.tile([128, 512], F32, tag="pb", name="pd2")
                d2sb = tactp.tile([128, 2, NN2], BF16, tag="d2", name="d2sb")
                for mc in range(2):
                    for kc in range(2):
                        nc.tensor.matmul(
                            pd2[:, npair * mc : npair * mc + npair],
                            dw2_t[:, kc, mc * 128 : mc * 128 + 128],
                            cx.d1sb[:, kc, :npair],
                            start=(kc == 0), stop=False,
                        )
                    nc.tensor.matmul(
                        pd2[:, npair * mc : npair * mc + npair],
                        brow_t[:, 10 + mc, :],
                        ones_t[:, :npair],
                        start=False, stop=True,
                    )
                nc.scalar.activation(
                    d2sb[:, :, :npair],
                    pd2[:, : 2 * npair].rearrange("p (g c) -> p g c", g=2),
                    AF.Tanh,
                )
                pd3 = ppb.tile([128, 512], F32, tag="pb", name="pd3")
                for kc in range(2):
                    nc.tensor.matmul(
                        pd3[:1, :npair], dw3_t[:, kc, :], d2sb[:, kc, :npair],
                        start=(kc == 0), stop=(kc == 1),
                    )
                outp = tactp.tile([1, NN2], F32, tag="outp", name="outp")
                nc.vector.tensor_scalar_add(
                    outp[:, :npair], pd3[:1, :npair],
                    bias_t[0:1, COL_DB3 : COL_DB3 + 1],
                )
                nc.sync.dma_start(out_d[:, cx.pcols], outp[:1, :npair])

            # Skewed software pipeline: pairs are processed in groups of
            # GSKEW; group g executes step s at tick g+1+s, so the STEPS
            # in-flight groups are each at a DIFFERENT step.  Phases are
            # emitted aligned across groups each tick, so every engine's
            # stream mixes independent work and no global step boundary
            # exists (the per-pair serial chain of one group is covered by
            # the other groups' phases).
            def make_cx(pr):
                cx = Ctx()
                cx.pr = pr
                cx.geos = [geom(k) for k in pr]
                cx.uni = len(pr) == 2 and cx.geos[0] == cx.geos[1]
                cx.col0 = NN * pr[0]
                cx.npair = sum(g[0] for g in cx.geos)
                cx.pcols = slice(cx.col0, cx.col0 + cx.npair)
                return cx

            allpairs = list(pairs) * repeat
            groups = [
                allpairs[i : i + GSKEW] for i in range(0, len(allpairs), GSKEW)
            ]
            gcxs = {}
            ngroups = len(groups)
            for T in range(ngroups + STEPS + 1):
                if T < ngroups:
                    gcxs[T] = [make_cx(pr) for pr in groups[T]]
                    for cx in gcxs[T]:
                        ph_load(cx)
                    for cx in gcxs[T]:
                        ph_enc(cx)
                for ph in (ph_P, ph_m1, ph_m2, ph_w3, ph_gru, ph_xupd):
                    for g in range(max(0, T - STEPS), min(T, ngroups)):
                        s = T - 1 - g
                        if 0 <= s < STEPS:
                            for cx in gcxs[g]:
                                ph(cx, s)
                g_done = T - 1 - STEPS
                if 0 <= g_done < ngroups:
                    for cx in gcxs[g_done]:
                        ph_dec1(cx)
                    for cx in gcxs[g_done]:
                        ph_dec2(cx)
                    del gcxs[g_done]

    nc.compile()
    _NC_CACHE[key] = nc
    return nc


def bench_hw(inputs, repeats=(1, 3), n_iter=14):
    """Differential HW timing: wall-clock difference between NEFFs that run
    the message-passing phase R times (dispatch overhead cancels)."""
    from test import bench  # local harness helper

    in_maps = preprocess(inputs, CFG_FULL)
    res = {}
    for r in repeats:
        nc = build_nc(CFG_FULL, repeat=r)
        times, _ = bench(nc, in_maps, n_iter=n_iter)
        ts = np.sort(times)[: max(3, n_iter // 2)]
        res[r] = ts.mean()
        print(f"repeat={r}: min {times.min()*1e3:.3f} ms  "
              f"low-half-mean {ts.mean()*1e3:.3f} ms")
    rs = sorted(res)
    phase = (res[rs[-1]] - res[rs[0]]) / (rs[-1] - rs[0])
    print(f"block-phase time ≈ {phase*1e3:.3f} ms")
    return phase, res


def preprocess(inputs, cfg):
    bg, nblk, tailg, totblk, nnode = (
        cfg["bg"], cfg["nblk"], cfg["tailg"], cfg["totblk"], cfg["nnode"],
    )
    b = bg * NCORES
    obs = np.asarray(inputs["obs"], np.float32)
    edges = np.asarray(inputs["edges"], np.int64)

    # one-hot incidence per graph
    src = edges[:, 0, :]
    dst = edges[:, 1, :]
    loops = np.broadcast_to(np.arange(N, dtype=np.int64), (b, N))
    src_all = np.concatenate([src, loops], 1)  # [b, 25]
    dst_all = np.concatenate([dst, loops], 1)
    nod = np.arange(N, dtype=np.int64)
    Sg = (src_all[:, None, :] == nod[None, :, None]).astype(np.float32)  # [b,9,25]
    Dg = (dst_all[:, None, :] == nod[None, :, None]).astype(np.float32)  # [b,9,25]
    deg = Dg.sum(2)  # [b, 9] >= 1
    Dw = Dg.transpose(0, 2, 1) / deg[:, None, :]  # [b, 25, 9]

    SDt = np.zeros((NCORES, totblk, NN, 2 * NE), NPE4)
    DWf = np.zeros((NCORES, totblk, 384, NN), np.float32)
    Sg_ = Sg.reshape(NCORES, bg, N, EPG)
    Dg_ = Dg.reshape(NCORES, bg, N, EPG)
    Dw_ = Dw.reshape(NCORES, bg, EPG, N)
    nmain = nblk * GPB
    Sm = Sg_[:, :nmain].reshape(NCORES, nblk, GPB, N, EPG)
    Dm = Dg_[:, :nmain].reshape(NCORES, nblk, GPB, N, EPG)
    Wm = Dw_[:, :nmain].reshape(NCORES, nblk, GPB, EPG, N)
    for i in range(GPB):
        r = slice(N * i, N * i + N)
        c = slice(EPG * i, EPG * i + EPG)
        SDt[:, :nblk, r, c] = Dm[:, :, i]  # dst-gather one-hot
        SDt[:, :nblk, r, NE + EPG * i : NE + EPG * i + EPG] = Sm[:, :, i]
        DWf[:, :nblk, c, r] = Wm[:, :, i]
    for i in range(tailg):
        g = nmain + i
        r = slice(N * i, N * i + N)
        c = slice(EPG * i, EPG * i + EPG)
        SDt[:, nblk, r, c] = Dg_[:, g]
        SDt[:, nblk, r, NE + EPG * i : NE + EPG * i + EPG] = Sg_[:, g]
        DWf[:, nblk, c, r] = Dw_[:, g]
    DWt = DWf.reshape(NCORES, totblk, 3, 128, NN).astype(NPBF16)

    obsT = (
        obs.reshape(b, N, F_IN)
        .reshape(NCORES, nnode, F_IN)
        .transpose(0, 2, 1)
        .astype(NPBF16)
    )  # [8, 15, nnode]

    f32 = lambda x: np.asarray(x, np.float32)
    bf = lambda x: np.ascontiguousarray(f32(x)).astype(NPBF16)
    e4 = lambda x: np.ascontiguousarray(f32(x)).astype(NPE4)

    biases = np.zeros((128, NBIAS), np.float32)
    biases[:, COL_ENC] = f32(inputs["enc_b"])
    gru_bi = f32(inputs["gru_bi"])
    gru_bh = f32(inputs["gru_bh"])
    msg_b3 = f32(inputs["msg_b3"])
    gru_Wi = f32(inputs["gru_Wi"])
    for s in range(STEPS):
        b1 = f32(inputs["msg_b1"][s])
        b2 = f32(inputs["msg_b2"][s])
        for c in range(2):
            biases[:, COL_B1(s, c)] = b1[128 * c : 128 * (c + 1)]
            biases[:, COL_B2(s, c)] = b2[128 * c : 128 * (c + 1)]
        bip = gru_bi[s] + msg_b3[s] @ gru_Wi[s]  # fold msg_b3 into GRU input bias
        for g in range(2):
            biases[:, COL_BRZ(s, g)] = 0.5 * (
                bip[128 * g : 128 * (g + 1)] + gru_bh[s][128 * g : 128 * (g + 1)]
            )
        biases[:, COL_BHN(s)] = gru_bh[s][256:384]
        biases[:, COL_BIN(s)] = bip[256:384]
    db1 = f32(inputs["dec_b1"])
    db2 = f32(inputs["dec_b2"])
    for c in range(2):
        biases[:, COL_DB1(c)] = db1[128 * c : 128 * (c + 1)]
        biases[:, COL_DB2(c)] = db2[128 * c : 128 * (c + 1)]
    biases[0, COL_DB3] = float(f32(inputs["dec_b3"])[0])

    brow = np.zeros((1, 16, 128), np.float32)
    whm = f32(inputs["gru_Wh"]).copy()
    whm[:, :, 256:384] *= 0.5
    for s in range(STEPS):
        bsum = gru_bi[s] + msg_b3[s] @ gru_Wi[s] + gru_bh[s]
        for g in range(2):
            brow[0, 2 * s + g] = bsum[128 * g : 128 * (g + 1)]
        brow[0, 12 + s] = 0.5 * gru_bh[s][256:384]
    for c in range(2):
        brow[0, 8 + c] = db1[128 * c : 128 * (c + 1)]
        brow[0, 10 + c] = db2[128 * c : 128 * (c + 1)]


    shared = dict(
        encw=bf(inputs["enc_W"]),
        w1=bf(inputs["msg_W1"]),
        w2=e4(inputs["msg_W2"]),
        w3=e4(inputs["msg_W3"]),
        wi=bf(inputs["gru_Wi"]),
        wh=bf(whm),
        dw1=bf(inputs["dec_W1"]),
        dw2=bf(inputs["dec_W2"]),
        dw3=bf(inputs["dec_W3"]),
        biases=biases,
        brow=brow.astype(NPBF16),
    )
    SDp = np.zeros((NCORES, totblk, NN, 2, NEP), NPE4)
    SDv = SDt.reshape(NCORES, totblk, NN, 2, NE)
    SDp[:, :, :, :, :NE] = SDv
    in_maps = []
    for c in range(NCORES):
        m = dict(shared)
        m["obsT"] = np.ascontiguousarray(obsT[c])
        m["sdt"] = np.ascontiguousarray(SDp[c])
        m["dwt"] = np.ascontiguousarray(DWt[c])
        in_maps.append(m)
    return in_maps


LAST_EXEC_NS = None
TRACE = False


def _run(inputs, cfg):
    global LAST_EXEC_NS
    nc = build_nc(cfg)
    in_maps = preprocess(inputs, cfg)
    res = run_bass_kernel_spmd(
        nc, in_maps, core_ids=list(range(NCORES)), trace=TRACE
    )
    LAST_EXEC_NS = res.exec_time_ns
    bg = cfg["bg"]
    outs = [np.asarray(res.results[c]["out"], np.float32).reshape(bg, N) for c in range(NCORES)]
    full = np.concatenate(outs, 0)  # [B, 9]
    return np.ascontiguousarray(full[:, :8])


def kernel(**inputs) -> np.ndarray:
    return _run(inputs, CFG_FULL)

